# revision 4
# baseline (speedup 1.0000x reference)
"""Trainium2 Bass kernel for nn_NeuralRDE (Tsit5 neural RDE forward pass), v3.

v2 + final tanh chunk split (384+128) to shorten the einsum tail.

v1 + persistent-bias mm3: 7 PSUM banks hold b2 + W2 @ h2acc permanently and
each f-eval streams W2 against dh = fp16(h2 - h2acc) (GPSIMD maintains
h2acc += dh in fp32, so the accumulated state tracks h2 to one fp16
rounding - no drift). This removes the per-f-eval rank-1 bias matmuls for
7/8 of the mm3 columns (~4096 -> ~1536 bias cycles). The first 512 asb
columns run conventionally through a 192-col slot in bank 7 (3 waves),
which also hosts ps12/ps2/psk.
"""

import sys

sys.path.insert(0, "/opt/trn_rl_repo")
import numpy as np

# ---------------- problem constants (hardcoded from the spec) ----------------
B, NI, LS, D, H, WID, LAB = 512, 256, 64, 8, 128, 256, 10
L = LS - 1  # 63
NSTEPS = 256
NCORES = 8
BL = B // NCORES  # 64 batch rows per core
QL = 32  # number of l-pair tiles (l padded 63 -> 64)

C2, C3, C4, C5, C6 = 0.161, 0.327, 0.9, 0.9800255409045097, 1.0
A21 = 0.161
A31, A32 = -0.008480655492356989, 0.335480655492357
A41, A42, A43 = 2.8971530571054935, -6.359448489975075, 4.3622954328695815
A51, A52, A53, A54 = 5.325864828439257, -11.748883564062828, 7.4955393428898365, -0.09249506636175525
A61, A62, A63, A64, A65 = 5.86145544294642, -12.92096931784711, 8.159367898576159, -0.071584973281401, -0.028269050394068383
B1_, B2_, B3_, B4_, B5_, B6_ = 0.09646076681806523, 0.01, 0.4798896504144996, 1.379008574103742, -3.290069515436081, 2.324710524099774

ACOEF = {
    2: [A21],
    3: [A31, A32],
    4: [A41, A42, A43],
    5: [A51, A52, A53, A54],
    6: [A61, A62, A63, A64, A65],
}
BCOEF = [B1_, B2_, B3_, B4_, B5_, B6_]

# conv waves over asb cols [0:512): (start, width) on the 192-col slot
WAVES = [(0, 192), (192, 192), (384, 128)]
# persistent banks b cover asb cols [512+512b, ...), 512 wide each
PBANKS = [(512 + 512 * b, 512) for b in range(7)]
# tanh chunks (asb ranges) in ACT emission order, with their sem_mm3 wait
# counts (PE inc order: wA=1,b0=2,b1=3,wB=4,b2=5,b3=6,wC=7,b4=8,b5=9,b6=10)
TANH_CHUNKS = [
    (0, 192, 1), (192, 384, 4), (384, 512, 7),        # conv waves
    (512, 1536, 3), (1536, 2560, 6), (2560, 3584, 9),
    (3584, 3968, 10), (3968, 4096, 10),   # last persistent bank split so the
    # final einsum tiles wait on a 128-col tanh, shortening the stage tail
]
# einsum q-tile q (asb cols [q*128,(q+1)*128)) -> # tanh chunks that must be
# done (1-based index in ACT emission order) = first chunk whose end >= q_end
Q_TANH_NEED = []
for q in range(QL):
    qe = (q + 1) * 128
    need = 0
    for ci, (a, b, _w) in enumerate(TANH_CHUNKS):
        if b >= qe and a <= q * 128:
            need = ci + 1
            break
    else:
        # spans chunk boundary within conv region: need both
        for ci, (a, b, _w) in enumerate(TANH_CHUNKS):
            if b >= qe:
                need = ci + 1
                break
    Q_TANH_NEED.append(need)


# ---------------- device kernel builder ----------------
def build_nc(nsteps=NSTEPS):
    import concourse.bass as bass
    import concourse.mybir as mybir
    from contextlib import ExitStack

    f16, f32 = mybir.dt.float16, mybir.dt.float32
    Silu = mybir.ActivationFunctionType.Silu
    Tanh = mybir.ActivationFunctionType.Tanh
    ADD, MUL = mybir.AluOpType.add, mybir.AluOpType.mult
    SUB = mybir.AluOpType.subtract

    dtv = np.float32(1.0 / NSTEPS)

    def cf(c):
        return float(np.float32(dtv * np.float32(c)))

    nc = bass.Bass()

    w0t_d = nc.declare_dram_parameter("w0t", [128, 256], f16, False)
    b0c_d = nc.declare_dram_parameter("b0c", [128, 4], f32, False)
    w1t_d = nc.declare_dram_parameter("w1t", [128, 512], f16, False)
    w2m_d = nc.declare_dram_parameter("w2m", [128, 16384], f16, False)
    b2m_d = nc.declare_dram_parameter("b2m", [128, 4096], f16, False)
    gall_d = nc.declare_dram_parameter("gall", [128, QL * nsteps], f32, False)
    y0_d = nc.declare_dram_parameter("y0", [128, 64], f32, False)
    cst_d = nc.declare_dram_parameter("cst", [128, 256], f16, False)
    yf_d = nc.declare_dram_parameter("yf", [128, 64], f32, True)

    pe, act, dve, sync = nc.tensor, nc.scalar, nc.vector, nc.sync
    gp = nc.gpsimd

    with ExitStack() as _es:
        ec = _es.enter_context
        # ---- SBUF ----
        w0t = ec(nc.sbuf_tensor("w0t_s", [128, 256], f16))
        b0c = ec(nc.sbuf_tensor("b0c_s", [128, 4], f32))
        w1t = ec(nc.sbuf_tensor("w1t_s", [128, 512], f16))
        w2m = ec(nc.sbuf_tensor("w2m_s", [128, 16384], f16))
        b2m = ec(nc.sbuf_tensor("b2m_s", [128, 4096], f16))
        gall = ec(nc.sbuf_tensor("gall_s", [128, QL * nsteps], f32))
        cst = ec(nc.sbuf_tensor("cst_s", [128, 256], f16))
        ysb = ec(nc.sbuf_tensor("ysb", [128, 64], f32))
        y16 = ec(nc.sbuf_tensor("y16", [128, 64], f16))
        pbuf = ec(nc.sbuf_tensor("pbuf", [128, 64], f32))
        ksb = ec(nc.sbuf_tensor("ksb", [128, 384], f32))
        h1 = ec(nc.sbuf_tensor("h1", [128, 128], f16))
        h2 = ec(nc.sbuf_tensor("h2", [128, 128], f16))
        h2acc = ec(nc.sbuf_tensor("h2acc", [128, 128], f32))
        dh0 = ec(nc.sbuf_tensor("dh0", [128, 128], f16))
        dh1 = ec(nc.sbuf_tensor("dh1", [128, 128], f16))
        asb = ec(nc.sbuf_tensor("asb", [128, 4096], f16))
        g0 = ec(nc.sbuf_tensor("g0", [128, QL * 64], f16))
        g1 = ec(nc.sbuf_tensor("g1", [128, QL * 64], f16))
        # ---- PSUM: 8 banks ----
        psA = ec(nc.psum_tensor("psA", [128, 3584], f32))   # banks 0-6 persistent
        psB = ec(nc.psum_tensor("psB", [128, 512], f32))    # bank 7 shared
        # ---- semaphores ----
        dma_sem = ec(nc.semaphore("dma_sem"))
        sem_y16 = ec(nc.semaphore("sem_y16"))
        sem_h = ec(nc.semaphore("sem_h"))
        sem_mm12 = ec(nc.semaphore("sem_mm12"))
        sem_mm3 = ec(nc.semaphore("sem_mm3"))
        sem_tanh = ec(nc.semaphore("sem_tanh"))
        sem_eins = ec(nc.semaphore("sem_eins"))
        sem_kev = ec(nc.semaphore("sem_kev"))
        sem_g = ec(nc.semaphore("sem_g"))
        sem_ord = ec(nc.semaphore("sem_ord"))
        sem_dh = ec(nc.semaphore("sem_dh"))
        sem_dacc = ec(nc.semaphore("sem_dacc"))

        G = [g0, g1]
        DH = [dh0, dh1]
        I_stack = cst[:, 0:64]
        bmask = cst[0:2, 128:256]
        zrow = b2m[2:3, :]  # all-zero f16 row (b2m rows >= 2 are zero)

        # psB col map
        SLOT = 0        # conv slot [0:192)
        P12 = 192       # ps12 [192:320)
        P2 = 320        # ps2  [320:448)
        PK = 448        # psk  [448:512)

        # ---------------- prologue ----------------
        n_dma = 0
        for dst, src in (
            (w0t, w0t_d), (b0c, b0c_d), (w1t, w1t_d), (w2m, w2m_d),
            (b2m, b2m_d), (gall, gall_d), (ysb, y0_d), (cst, cst_d),
        ):
            sync.dma_start(dst[:, :], src[:, :]).then_inc(dma_sem, 16)
            n_dma += 1
        sync.wait_ge(dma_sem, 16 * n_dma)
        nc.all_engine_barrier()

        # PE: persistent banks <- bias; bank 7 <- zeros (marks pending)
        for b, (c0, w) in enumerate(PBANKS):
            pe.matmul(psA[:, b * 512:b * 512 + w], bmask, b2m[0:2, c0:c0 + w],
                      start=True, stop=True)
        # zero bank 7: any stationary x zero moving (cst[0, 64:128) is zero)
        for j in range(8):
            pe.matmul(psB[:, j * 64:(j + 1) * 64], cst[0:1, 128:256],
                      cst[0:1, 64:128], start=True, stop=True)
        # h2acc = 0 on DVE (no GPSIMD anywhere: its runtime path crashes)
        dve.memset(h2acc[:, :], 0)
        dve.memset(dh0[:, :], 0)
        dve.memset(dh1[:, :], 0)
        # DVE: G buffers for step 0, pbuf, y16
        dve.tensor_copy(y16[:, :], ysb[:, :]).then_inc(sem_y16)
        dve.tensor_copy(pbuf[:, :], ysb[:, :])
        for q in range(QL):
            ins = dve.tensor_scalar_mul(g1[:, q * 64:(q + 1) * 64], I_stack, gall[:, q:q + 1])
        ins.then_inc(sem_g)
        for q in range(QL):
            ins = dve.tensor_scalar_mul(g0[:, q * 64:(q + 1) * 64], I_stack, gall[:, q:q + 1])
        ins.then_inc(sem_g)
        nc.all_engine_barrier()

        # ---------------- main loop ----------------
        g_queue = []
        ordc = [0]

        def ord_inc(ins):
            ins.then_inc(sem_ord)
            ordc[0] += 1

        def g_fill():
            if g_queue:
                dst, col = g_queue.pop(0)
                gins = dve.tensor_scalar_mul(dst, I_stack, col)
                if not g_queue:
                    gins.then_inc(sem_g)

        for i in range(nsteps):
            for s in range(1, 7):
                fe = i * 6 + (s - 1)
                dh = DH[fe % 2]

                # ======== PE stream ========
                pe.wait_ge(sem_y16, fe + 1)
                if fe >= 1:
                    pe.wait_ge(sem_h, 4 * (fe - 1) + 2)   # silu1(fe-1) read ps12
                # mm1 into psB[P12:P12+128): bank was cleared by wave C(fe-1)
                pe.matmul(psB[:, P12:P12 + 64], w0t[:, 0:128], y16[:, :],
                          start=False, stop=False, skip_group_check=True)
                pe.matmul(psB[:, P12 + 64:P12 + 128], w0t[:, 128:256], y16[:, :],
                          start=False, stop=False, skip_group_check=True).then_inc(sem_mm12)
                # mm2 into psB[P2:P2+128)
                pe.wait_ge(sem_h, fe * 4 + 1)
                pe.matmul(psB[:, P2:P2 + 64], w1t[:, 0:128], h1[:, 0:64],
                          start=False, stop=False, skip_group_check=True)
                pe.matmul(psB[:, P2 + 64:P2 + 128], w1t[:, 128:256], h1[:, 0:64],
                          start=False, stop=False, skip_group_check=True)
                pe.wait_ge(sem_h, fe * 4 + 2)
                pe.matmul(psB[:, P2:P2 + 64], w1t[:, 256:384], h1[:, 64:128],
                          start=False, stop=False, skip_group_check=True)
                pe.matmul(psB[:, P2 + 64:P2 + 128], w1t[:, 384:512], h1[:, 64:128],
                          start=False, stop=False, skip_group_check=True).then_inc(sem_mm12)

                # mm3: interleave conv waves (full h2, bias per wave) with
                # persistent banks (dh stream, no bias)
                def conv_wave(widx):
                    a0, w = WAVES[widx]
                    # psk consumed (wave A clears the whole bank incl psk)
                    if widx == 0:
                        pe.wait_ge(sem_kev, fe)
                        pe.wait_ge(sem_h, fe * 4 + 4)  # silu2 done reading ps2
                        if fe >= 1:
                            pe.wait_ge(sem_tanh, (fe - 1) * 8 + 3)  # cC(fe-1)
                    else:
                        pe.wait_ge(sem_tanh, fe * 8 + widx)  # prev wave's tanh
                    pe.matmul(psB[:, SLOT:SLOT + w], bmask, b2m[0:2, a0:a0 + w],
                              start=True, stop=True)
                    pe.matmul(psB[0:64, SLOT:SLOT + w], h2[:, 0:64], w2m[:, a0:a0 + w],
                              start=False, stop=False, tile_position=(0, 0), skip_group_check=True)
                    pe.matmul(psB[64:128, SLOT:SLOT + w], h2[:, 0:64], w2m[:, 8192 + a0:8192 + a0 + w],
                              start=False, stop=False, tile_position=(0, 64), skip_group_check=True)
                    pe.matmul(psB[0:64, SLOT:SLOT + w], h2[:, 64:128], w2m[:, 4096 + a0:4096 + a0 + w],
                              start=False, stop=False, tile_position=(0, 0), skip_group_check=True)
                    pe.matmul(psB[64:128, SLOT:SLOT + w], h2[:, 64:128], w2m[:, 12288 + a0:12288 + a0 + w],
                              start=False, stop=False, tile_position=(0, 64), skip_group_check=True).then_inc(sem_mm3)

                def pbank(b):
                    c0, w = PBANKS[b]
                    po = b * 512
                    # re-accumulate onto cols read by tanh(fe-1)
                    if fe >= 1:
                        pe.wait_ge(sem_tanh, (fe - 1) * 8 + 4 + (b // 2 if b < 6 else 4))
                    pe.wait_ge(sem_dh, fe * 2 + 1)
                    pe.matmul(psA[0:64, po:po + w], dh[:, 0:64], w2m[:, c0:c0 + w],
                              start=False, stop=False, tile_position=(0, 0), skip_group_check=True)
                    pe.matmul(psA[64:128, po:po + w], dh[:, 0:64], w2m[:, 8192 + c0:8192 + c0 + w],
                              start=False, stop=False, tile_position=(0, 64), skip_group_check=True)
                    pe.wait_ge(sem_dh, fe * 2 + 2)
                    pe.matmul(psA[0:64, po:po + w], dh[:, 64:128], w2m[:, 4096 + c0:4096 + c0 + w],
                              start=False, stop=False, tile_position=(0, 0), skip_group_check=True)
                    pe.matmul(psA[64:128, po:po + w], dh[:, 64:128], w2m[:, 12288 + c0:12288 + c0 + w],
                              start=False, stop=False, tile_position=(0, 64), skip_group_check=True).then_inc(sem_mm3)

                conv_wave(0)
                pbank(0)
                pbank(1)
                conv_wave(1)
                pbank(2)
                pbank(3)
                conv_wave(2)
                pbank(4)
                pbank(5)
                pbank(6)

                # einsum -> psk (bank 7, pending from wave C's bias clear)
                pe.wait_ge(sem_g, i + 1 if s == 1 else i + 2)
                gbuf = G[(i - 1) % 2] if s == 1 else G[i % 2]
                last_need = -1
                for q in range(QL):
                    if Q_TANH_NEED[q] != last_need:
                        last_need = Q_TANH_NEED[q]
                        pe.wait_ge(sem_tanh, fe * 8 + last_need)
                    ins = pe.matmul(psB[:, PK:PK + 64], asb[:, q * 128:(q + 1) * 128],
                                    gbuf[:, q * 64:(q + 1) * 64],
                                    start=False, stop=False, skip_group_check=True)
                ins.then_inc(sem_eins)

                # ======== ACT stream ========
                act.wait_ge(sem_mm12, fe * 2 + 1)
                act.activation(h1[:, 0:64], psB[:, P12:P12 + 64], Silu, bias=b0c[:, 0:1]).then_inc(sem_h)
                act.activation(h1[:, 64:128], psB[:, P12 + 64:P12 + 128], Silu, bias=b0c[:, 1:2]).then_inc(sem_h)
                act.wait_ge(sem_mm12, fe * 2 + 2)
                act.activation(h2[:, 0:64], psB[:, P2:P2 + 64], Silu, bias=b0c[:, 2:3]).then_inc(sem_h)
                act.activation(h2[:, 64:128], psB[:, P2 + 64:P2 + 128], Silu, bias=b0c[:, 3:4]).then_inc(sem_h)
                for (ca, cb, wcnt) in TANH_CHUNKS:
                    act.wait_ge(sem_mm3, fe * 10 + wcnt)
                    if ca < 512:  # conv chunk from the bank-7 slot
                        src = psB[:, SLOT:SLOT + (cb - ca)]
                    else:
                        src = psA[:, ca - 512:cb - 512]
                    act.activation(asb[:, ca:cb], src, Tanh).then_inc(sem_tanh)

                # ======== GPSIMD stream: dh = f16(h2 - h2acc); h2acc += dh ====
                dve.wait_ge(sem_dacc, fe * 2)
                if fe >= 1:
                    dve.wait_ge(sem_mm3, (fe - 1) * 10 + 10)  # PE done with dh(fe-1)
                dve.wait_ge(sem_h, fe * 4 + 3)
                dve.tensor_sub(dh[:, 0:64], h2[:, 0:64], h2acc[:, 0:64]).then_inc(sem_dh)
                dve.wait_ge(sem_h, fe * 4 + 4)
                dve.tensor_sub(dh[:, 64:128], h2[:, 64:128], h2acc[:, 64:128]).then_inc(sem_dh)
                dve.wait_ge(sem_dh, fe * 2 + 2)
                dve.tensor_add(h2acc[:, 0:64], h2acc[:, 0:64], dh[:, 0:64]).then_inc(sem_dacc)
                dve.tensor_add(h2acc[:, 64:128], h2acc[:, 64:128], dh[:, 64:128]).then_inc(sem_dacc)

                # ======== DVE stream ========
                dve.wait_ge(sem_eins, fe + 1)
                if s == 1 and i + 1 < nsteps:
                    g_queue[:] = [(G[(i + 1) % 2][:, q * 64:(q + 1) * 64],
                                   gall[:, (i + 1) * QL + q:(i + 1) * QL + q + 1]) for q in range(QL)]
                gbudget = 7

                cc = cf(ACOEF[s + 1][s - 1] if s < 6 else BCOEF[5])
                dve.wait_ge(sem_ord, ordc[0])
                dve.scalar_tensor_tensor(y16[:, :], psB[:, PK:PK + 64], cc, pbuf[:, :],
                                         op0=MUL, op1=ADD).then_inc(sem_y16)
                if s == 6:
                    ord_inc(dve.scalar_tensor_tensor(ysb[:, :], psB[:, PK:PK + 64], cc, pbuf[:, :],
                                                     op0=MUL, op1=ADD))
                dve.tensor_copy(ksb[:, (s - 1) * 64:s * 64], psB[:, PK:PK + 64]).then_inc(sem_kev)
                if s <= 4:
                    coefs = [cf(c) for c in ACOEF[s + 2][:s]]
                elif s == 5:
                    coefs = [cf(c) for c in BCOEF[:5]]
                else:
                    coefs = None
                if coefs is not None:
                    dve.wait_ge(sem_kev, fe + 1)
                    for j, c in enumerate(coefs):
                        dve.wait_ge(sem_ord, ordc[0])
                        srcb = ysb if j == 0 else pbuf
                        ord_inc(dve.scalar_tensor_tensor(pbuf[:, :], ksb[:, j * 64:(j + 1) * 64],
                                                         c, srcb[:, :], op0=MUL, op1=ADD))
                        if gbudget > 0:
                            g_fill()
                            gbudget -= 1
                else:
                    dve.wait_ge(sem_ord, ordc[0])
                    ord_inc(dve.tensor_copy(pbuf[:, :], ysb[:, :]))
                while gbudget > 0 and g_queue:
                    g_fill()
                    gbudget -= 1

        # ---------------- epilogue ----------------
        nc.all_engine_barrier()
        sync.dma_start(yf_d[:, :], ysb[:, :]).then_inc(dma_sem, 16)
        sync.wait_ge(dma_sem, 16 * (n_dma + 1))

    return nc


# ---------------- host-side input prep (same layouts as v1) ----------------
def _searchsorted_sched(ts, intervals, nsteps=NSTEPS):
    dt = np.float32((ts[-1] - ts[0]) / np.float32(NSTEPS))
    cs = [np.float32(c) for c in (0.0, C2, C3, C4, C5, C6)]
    t = np.float32(ts[0])
    sched = np.zeros((nsteps, 6), dtype=np.int64)
    for i in range(nsteps):
        for s, c in enumerate(cs):
            tau = t if s == 0 else np.float32(t + c * dt)
            idx = int(np.searchsorted(intervals, tau, side="left"))
            sched[i, s] = min(max(idx, 0), NI - 1)
        t = np.float32(t + dt)
    return sched


def prep_core_inputs(inputs, core, nsteps=NSTEPS, sched=None):
    f16 = np.float16
    W0, b0, W1, b1, W2, b2 = (np.asarray(inputs[k], np.float32) for k in ("W0", "b0", "W1", "b1", "W2", "b2"))
    l1w, l1b = np.asarray(inputs["l1w"], np.float32), np.asarray(inputs["l1b"], np.float32)
    logsig, x0 = np.asarray(inputs["logsig"], np.float32), np.asarray(inputs["x0"], np.float32)
    if sched is None:
        sched = _searchsorted_sched(np.asarray(inputs["ts"], np.float32),
                                    np.asarray(inputs["intervals"], np.float32), nsteps)
    bs = slice(core * BL, (core + 1) * BL)

    w0t = np.ascontiguousarray(W0.T).astype(f16)
    b0c = np.zeros((128, 4), np.float32)
    b0c[:, 0] = b0[0:128]
    b0c[:, 1] = b0[128:256]
    b0c[:, 2] = b1[0:128]
    b0c[:, 3] = b1[128:256]

    W1T = W1.T
    w1t = np.concatenate([W1T[0:128, :], W1T[128:256, :]], axis=1).astype(f16)

    W2p = np.zeros((H, 64, WID), np.float32)
    W2p[:, :L, :] = W2.reshape(H, L, WID)
    w2m = np.zeros((128, 16384), np.float32)
    for c in (0, 1):
        for kt in (0, 1):
            blk = W2p[:, c::2, kt * 128:(kt + 1) * 128]
            w2m[:, c * 8192 + kt * 4096:c * 8192 + (kt + 1) * 4096] = \
                blk.transpose(2, 1, 0).reshape(128, 4096)
    w2m = w2m.astype(f16)

    b2p = np.zeros((H, 64), np.float32)
    b2p[:, :L] = b2.reshape(H, L)
    b2m = np.zeros((128, 4096), np.float32)
    for c in (0, 1):
        b2m[c, :] = b2p[:, c::2].T.reshape(-1)
    b2m = b2m.astype(f16)

    assert all(np.array_equal(sched[:, s], sched[:, 1]) for s in range(2, 6)), "irregular schedule"
    exp_prev = np.concatenate([[sched[0, 1]], sched[:-1, 1]])
    assert np.array_equal(sched[:, 0], exp_prev), "irregular stage-1 schedule"

    gall = np.zeros((128, QL * nsteps), np.float32)
    for i in range(nsteps):
        glp = np.zeros((BL, 64), np.float32)
        glp[:, :L] = logsig[bs, sched[i, 1], 1:]
        for c in (0, 1):
            gall[c * 64:(c + 1) * 64, i * QL:(i + 1) * QL] = glp[:, c::2]

    y0 = (x0[bs] @ l1w.T + l1b).astype(np.float32).T.copy()

    cst = np.zeros((128, 256), np.float32)
    for p in range(128):
        cst[p, p % 64] = 1.0
    cst[0, 128:192] = 1.0
    cst[1, 192:256] = 1.0
    cst = cst.astype(f16)

    return dict(w0t=w0t, b0c=b0c, w1t=w1t, w2m=w2m, b2m=b2m,
                gall=gall, y0=np.ascontiguousarray(y0), cst=cst)


def finish_head(yf_list, inputs):
    l2w, l2b = np.asarray(inputs["l2w"], np.float32), np.asarray(inputs["l2b"], np.float32)
    ys = [yf.T for yf in yf_list]
    y = np.concatenate(ys, axis=0)
    logits = y @ l2w.T + l2b
    e = np.exp(logits - logits.max(axis=-1, keepdims=True))
    return (e / e.sum(axis=-1, keepdims=True)).astype(np.float32)


_NC_CACHE = {}


def kernel(**inputs):
    from concourse.bass_utils import run_bass_kernel_spmd

    nsteps = NSTEPS
    if nsteps not in _NC_CACHE:
        _NC_CACHE[nsteps] = build_nc(nsteps)
    nc = _NC_CACHE[nsteps]

    sched = _searchsorted_sched(np.asarray(inputs["ts"], np.float32),
                                np.asarray(inputs["intervals"], np.float32), nsteps)
    in_maps = [prep_core_inputs(inputs, ci, nsteps, sched) for ci in range(NCORES)]
    res = run_bass_kernel_spmd(nc, in_maps, list(range(NCORES)))
    yf_list = [np.asarray(res.results[ci]["yf"], np.float32) for ci in range(NCORES)]
    return finish_head(yf_list, inputs)
